# revision 2
# baseline (speedup 1.0000x reference)
"""ChebConv (K=2, 3 hops) GNN layer on 8 Trainium2 NeuronCores.

    out = relu( sum_i  A_i @ (x @ W_i) + b_i )          (reference order)
        = relu( sum_i (A_i @ x) @ W_i  +  sum_i b_i )   (this kernel)

Strategy (per the node-sharding hint):
  * Destination nodes are sharded across 8 cores (6250 rows each); each core
    owns all edges targeting its rows and the full x (input replication, no
    collectives needed).
  * Host-side index preprocessing only: per (core, hop) edges are bucketed by
    128-node destination window and by src < / >= 32768 (dma_gather indices
    are int16), padded to 128-edge chunks.
  * On device, per window/hop: dma_gather pulls the edges' source rows of x
    from HBM (512B rows, 4 SWDGE queues); a one-instruction DVE tensor_scalar
    builds the weighted one-hot scatter matrix S^T[e, v] = (iota==dst_rel)*w;
    the PE contracts over edges: aggT[f, v] += Xg[e, f]^T. S^T[e, v].
  * A second per-hop matmul applies W_i (aggT as lhsT directly), accumulating
    the three hops in PSUM; the summed bias enters as a rank-1 matmul; ACT
    applies the ReLU; results stream out per window.
"""

import numpy as np

N_NODES = 50000
N_EDGES = 800000
D = 128          # feature dim
U = 128          # units
NH = 3           # hops
NC = 8           # cores
RPC = N_NODES // NC   # rows (dst nodes) per core = 6250
P = 128
NW = (RPC + P - 1) // P   # dst windows per core = 49
SPLIT = 32768    # int16-safe src index split
NQ = 4           # SWDGE queues

_cache = {}


def _preprocess(edge_index, edge_weight):
    """Bucket edges by (core, hop, window, src-half); build per-core device
    metadata arrays. Returns (CLO, CHI, per-core list of dicts)."""
    src_all = np.asarray(edge_index[:, 0], dtype=np.int64)
    dst_all = np.asarray(edge_index[:, 1], dtype=np.int64)
    ew_all = np.asarray(edge_weight, dtype=np.float32)

    # pass 1: segment counts for capacity sizing
    segs = []  # (c, i) -> dict with sorted arrays + boundaries
    max_lo = 0
    max_hi = 0
    for c in range(NC):
        for i in range(NH):
            dsts = dst_all[i]
            sel = (dsts >= c * RPC) & (dsts < (c + 1) * RPC)
            ld = (dsts[sel] - c * RPC).astype(np.int32)
            s = src_all[i][sel].astype(np.int32)
            ww = ew_all[i][sel]
            win = ld >> 7
            half = (s >= SPLIT).astype(np.int32)
            key = win * 2 + half
            order = np.argsort(key, kind="stable")
            key_s = key[order]
            bounds = np.searchsorted(key_s, np.arange(NW * 2 + 1))
            counts = bounds[1:] - bounds[:-1]
            max_lo = max(max_lo, int(counts[0::2].max()))
            max_hi = max(max_hi, int(counts[1::2].max()))
            segs.append((c, i, ld[order], s[order], ww[order], bounds))

    CLO = (max_lo + P - 1) // P
    CHI = max(1, (max_hi + P - 1) // P)
    ncalls_per_hop = NW * 2
    tot_chunks = NH * NW * (CLO + CHI)

    cores = []
    for c in range(NC):
        cores.append({
            "idx": np.zeros((128, tot_chunks * 8), dtype=np.int16),
            "dstrel": np.zeros((128, tot_chunks), dtype=np.float32),
            "wgt": np.zeros((128, tot_chunks), dtype=np.float32),
        })

    for (c, i, ld, s, ww, bounds) in segs:
        arr = cores[c]
        for w in range(NW):
            for h, cap in ((0, CLO), (1, CHI)):
                b0 = bounds[w * 2 + h]
                b1 = bounds[w * 2 + h + 1]
                n = b1 - b0
                # call index within the kernel's (w, i, h) loop order
                k = (w * NH + i) * 2 + h
                nlo = (k + 1) // 2
                nhi = k // 2
                chunk_off = nlo * CLO + nhi * CHI
                if n == 0:
                    continue
                cap_e = cap * P
                idx16 = np.zeros(cap_e, dtype=np.int16)
                idx16[:n] = (s[b0:b1] - h * SPLIT).astype(np.int16)
                # slot j -> partition j%16 (x8 replicas), col j//16
                wrapped = idx16.reshape(cap * 8, 16).T  # [16, cap*8]
                arr["idx"][:, chunk_off * 8:(chunk_off + cap) * 8] = np.tile(
                    wrapped, (8, 1))
                dr = np.zeros(cap_e, dtype=np.float32)
                dr[:n] = (ld[b0:b1] - w * P).astype(np.float32)
                wv = np.zeros(cap_e, dtype=np.float32)
                wv[:n] = ww[b0:b1]
                # slot j -> partition j%128, col j//128
                arr["dstrel"][:, chunk_off:chunk_off + cap] = dr.reshape(cap, P).T
                arr["wgt"][:, chunk_off:chunk_off + cap] = wv.reshape(cap, P).T

    return CLO, CHI, cores


def _build(CLO, CHI):
    import concourse.bass as bass
    import concourse.mybir as mybir
    import concourse.tile as tile
    from concourse import bacc
    from concourse.library_config import mlp

    f32 = mybir.dt.float32
    tot_chunks = NH * NW * (CLO + CHI)

    nc = bacc.Bacc("TRN2", debug=False, num_devices=NC, num_swdge_queues=NQ)
    x_d = nc.dram_tensor("x", [N_NODES, D], f32, kind="ExternalInput")
    idx_d = nc.dram_tensor("idx", [128, tot_chunks * 8], mybir.dt.int16,
                           kind="ExternalInput")
    dr_d = nc.dram_tensor("dstrel", [128, tot_chunks], f32, kind="ExternalInput")
    wg_d = nc.dram_tensor("wgt", [128, tot_chunks], f32, kind="ExternalInput")
    wk_d = nc.dram_tensor("kernelw", [NH, D, U], f32, kind="ExternalInput")
    b_d = nc.dram_tensor("bias3", [NH, U], f32, kind="ExternalInput")
    out_d = nc.dram_tensor("out", [RPC, U], f32, kind="ExternalOutput")

    with tile.TileContext(nc) as tc:
        nc.gpsimd.load_library(mlp)
        with (
            tc.tile_pool(name="meta", bufs=1) as meta,
            tc.tile_pool(name="gatlo", bufs=3) as gatlo,
            tc.tile_pool(name="gathi", bufs=3) as gathi,
            tc.tile_pool(name="st", bufs=6) as stp,
            tc.tile_pool(name="aggsb", bufs=3) as aggsb,
            tc.tile_pool(name="outsb", bufs=3) as outsb,
            tc.tile_pool(name="psag", bufs=4, space="PSUM") as psag,
            tc.tile_pool(name="psout", bufs=2, space="PSUM") as psout,
            tc.tile_pool(name="psmisc", bufs=1, space="PSUM") as psmisc,
        ):
            idx_sb = meta.tile([128, tot_chunks * 8], mybir.dt.int16)
            nc.sync.dma_start(idx_sb[:], idx_d[:])
            dr_sb = meta.tile([128, tot_chunks], f32)
            nc.sync.dma_start(dr_sb[:], dr_d[:])
            wg_sb = meta.tile([128, tot_chunks], f32)
            nc.sync.dma_start(wg_sb[:], wg_d[:])

            wk_sb = meta.tile([128, NH * U], f32)
            for i in range(NH):
                nc.sync.dma_start(wk_sb[:, i * U:(i + 1) * U], wk_d[i])
            b3_sb = meta.tile([NH, U], f32)
            nc.sync.dma_start(b3_sb[:], b_d[:])

            iota_sb = meta.tile([128, P], f32)
            nc.gpsimd.iota(iota_sb[:], pattern=[[1, P]], base=0,
                           channel_multiplier=0,
                           allow_small_or_imprecise_dtypes=True)

            ones3 = meta.tile([NH, 1], f32)
            nc.gpsimd.memset(ones3[:], 1.0)
            ones_row = meta.tile([1, P], f32)
            nc.gpsimd.memset(ones_row[:], 1.0)

            # bias_sum[u] = sum_i bias[i, u]  (rank-3 matmul)
            bs_ps = psmisc.tile([1, U], f32, space="PSUM")
            nc.tensor.matmul(bs_ps[:], lhsT=ones3[:], rhs=b3_sb[:],
                             start=True, stop=True)
            bsum_sb = meta.tile([1, U], f32)
            nc.vector.tensor_copy(bsum_sb[:], bs_ps[:])

            call_idx = 0
            chunk_off = 0
            for w in range(NW):
                out_ps = psout.tile([P, U], f32, space="PSUM")
                for i in range(NH):
                    agg_ps = psag.tile([128, P], f32, space="PSUM")
                    n_chunks_wi = CLO + CHI
                    done = 0
                    for h, cap, pool in ((0, CLO, gatlo), (1, CHI, gathi)):
                        gt = pool.tile([128, cap, D], f32)
                        nc.gpsimd.dma_gather(
                            gt[:],
                            x_d[:] if h == 0 else x_d[SPLIT:, :],
                            idx_sb[:, chunk_off * 8:(chunk_off + cap) * 8],
                            cap * P,
                            cap * P,
                            D,
                            queue_num=call_idx % NQ,
                            single_packet=False,
                        )
                        call_idx += 1
                        for cc in range(cap):
                            st = stp.tile([128, P], f32)
                            col = chunk_off + cc
                            nc.vector.tensor_scalar(
                                st[:], iota_sb[:],
                                dr_sb[:, col:col + 1],
                                wg_sb[:, col:col + 1],
                                mybir.AluOpType.is_equal,
                                mybir.AluOpType.mult,
                            )
                            nc.tensor.matmul(
                                agg_ps[:], lhsT=gt[:, cc, :], rhs=st[:],
                                start=(done == 0),
                                stop=(done == n_chunks_wi - 1),
                            )
                            done += 1
                        chunk_off += cap
                    agg_sb = aggsb.tile([128, P], f32)
                    nc.vector.tensor_copy(agg_sb[:], agg_ps[:])
                    nc.tensor.matmul(
                        out_ps[:], lhsT=agg_sb[:],
                        rhs=wk_sb[:, i * U:(i + 1) * U],
                        start=(i == 0), stop=False,
                    )
                # += ones[v] * bias_sum[u]
                nc.tensor.matmul(out_ps[:], lhsT=ones_row[:], rhs=bsum_sb[:],
                                 start=False, stop=True)
                o_sb = outsb.tile([P, U], f32)
                nc.scalar.activation(o_sb[:], out_ps[:],
                                     mybir.ActivationFunctionType.Relu)
                rows = min(P, RPC - w * P)
                nc.sync.dma_start(out_d[w * P:w * P + rows, :], o_sb[:rows, :])

    nc.compile()
    return nc


def kernel(x, kernel, bias, edge_weight, edge_index):
    from concourse.bass_utils import run_bass_kernel_spmd

    x = np.ascontiguousarray(np.asarray(x, dtype=np.float32))
    wk = np.ascontiguousarray(np.asarray(kernel, dtype=np.float32))
    b = np.ascontiguousarray(np.asarray(bias, dtype=np.float32))

    CLO, CHI, cores = _preprocess(np.asarray(edge_index), edge_weight)

    key = (CLO, CHI)
    if key not in _cache:
        _cache[key] = _build(CLO, CHI)
    nc = _cache[key]

    in_maps = []
    for c in range(NC):
        in_maps.append({
            "x": x,
            "idx": cores[c]["idx"],
            "dstrel": cores[c]["dstrel"],
            "wgt": cores[c]["wgt"],
            "kernelw": wk,
            "bias3": b,
        })
    res = run_bass_kernel_spmd(nc, in_maps, core_ids=list(range(NC)))
    out = np.concatenate([res.results[c]["out"] for c in range(NC)], axis=0)
    return out
